# revision 14
# baseline (speedup 1.0000x reference)
"""TRN2 Bass kernel for nn_MultiHeadAttention_87119116632098 (GQA + RoPE + causal).

Sharding: batch (2) x kv-head-pair (4) = 8 cores, per the hint. Each core
computes Q/K/V projections for its 8 q-heads / 2 kv-heads, RoPE, causal
grouped attention, and a partial wo product (its 1024 wo input rows).
The host sums the 4 partial wo outputs per batch (the "all-reduce") and
reassembles the kv cache; both are exact fp32 adds / pure gathers.

All matmuls run as float32r (TRN2 reduced-precision fp32, ~13-bit
mantissa, 1 PE cycle/row at N>=256 vs 4 cycles/row for full fp32).
Measured end-to-end numerics vs the fp32 reference: ~1e-3 scale-relative.

Layouts (per core):
  qT/kT/vT   [D=4096, T=2048]  (host-transposed) -> moving/stationary chunks
  QT, KT     [head][128=headdim, T]   (projection output, transposed, roped)
  V          [head][128=t-chunk, 16, 128=headdim]  (natural)
  scores^T   [t_k=128, t_q=512] tiles; softmax without max-subtraction
             (scores are ~N(0,1.6), max ~10 -> exp is fp32-safe);
             denominators via ones-vector matmuls accumulated in PSUM,
             normalization via PE-broadcast of 1/denom.
  OT         [head][128=headdim, T]   (unnormalized attention out, f32r)
  out        [t=128, n=512] tiles = sum_e OT^T chunks @ wo chunks
"""
import os
import sys

sys.path.insert(0, "/opt/trn_rl_repo")

import numpy as np
from contextlib import ExitStack

import concourse.bass as bass
from concourse import bacc
import concourse.mybir as mybir
import concourse.tile as tile
from concourse.bass_utils import run_bass_kernel_spmd

B, T, D = 2, 2048, 4096
NQH, NKVH, HD = 32, 8, 128
GRP = NQH // NKVH              # 4
LQH, LKVH = 8, 2               # per-core q heads / kv heads
EQ = LQH * HD                  # 1024
EKV = LKVH * HD                # 256
DC = D // 128                  # 32 contraction chunks
TQW = 512                      # t_q tile width
NTQ = T // TQW                 # 4
KCW = 128                      # t_k chunk width
ROPE_BASE = 10000.0
SCALE = 1.0 / float(np.sqrt(HD))

F32 = mybir.dt.float32
F32R = mybir.dt.float32r

_CACHE = {}
LAST_RESULTS = None


def _build_nc():
    nc = bacc.Bacc("TRN2", target_bir_lowering=False, debug=True)

    qT = nc.dram_tensor("qT", [D, T], F32R, kind="ExternalInput")
    kT = nc.dram_tensor("kT", [D, T], F32R, kind="ExternalInput")
    vT = nc.dram_tensor("vT", [D, T], F32R, kind="ExternalInput")
    wqg = nc.dram_tensor("wqg", [D, EQ], F32R, kind="ExternalInput")
    wkg = nc.dram_tensor("wkg", [D, EKV], F32R, kind="ExternalInput")
    wvg = nc.dram_tensor("wvg", [D, EKV], F32R, kind="ExternalInput")
    wog = nc.dram_tensor("wog", [EQ, D], F32R, kind="ExternalInput")
    cosdt = nc.dram_tensor("cosdt", [HD, T], F32, kind="ExternalInput")
    sindt = nc.dram_tensor("sindt", [HD, T], F32, kind="ExternalInput")
    mask4 = nc.dram_tensor("mask4", [KCW, GRP, TQW], F32, kind="ExternalInput")
    ones_col = nc.dram_tensor("ones_col", [128, 1], F32R, kind="ExternalInput")
    ones_row = nc.dram_tensor("ones_row", [1, 128], F32R, kind="ExternalInput")
    identm = nc.dram_tensor("identm", [128, 128], F32, kind="ExternalInput")

    outp = nc.dram_tensor("outp", [T, D], F32, kind="ExternalOutput")
    kvout = nc.dram_tensor("kvout", [T, LKVH, 2 * HD], F32, kind="ExternalOutput")

    with tile.TileContext(nc) as tc, ExitStack() as ctx:
        consts = ctx.enter_context(tc.tile_pool(name="consts", bufs=1))
        ktv = ctx.enter_context(tc.tile_pool(name="ktv", bufs=1))

        ones_c = consts.tile([128, 1], F32R)
        nc.sync.dma_start(out=ones_c, in_=ones_col[:, :])
        ones_r = consts.tile([1, 128], F32R)
        nc.sync.dma_start(out=ones_r, in_=ones_row[:, :])
        ident = consts.tile([128, 128], F32)
        nc.sync.dma_start(out=ident, in_=identm[:, :])
        mask_t = consts.tile([KCW, GRP, TQW], F32)
        nc.sync.dma_start(out=mask_t, in_=mask4[:, :, :])

        KT = [ktv.tile([128, T], F32R, tag=f"KT{h}", name=f"KT{h}") for h in range(LKVH)]
        V = [ktv.tile([128, T // 128, HD], F32R, tag=f"V{h}", name=f"V{h}") for h in range(LKVH)]

        with tc.tile_pool(name="qtp", bufs=1) as qtp:
            tables_cm = tc.tile_pool(name="tables", bufs=1)
            tables = tables_cm.__enter__()
            cos_t = tables.tile([HD, T], F32)
            nc.sync.dma_start(out=cos_t, in_=cosdt[:, :])
            sin_t = tables.tile([HD, T], F32)
            nc.sync.dma_start(out=sin_t, in_=sindt[:, :])

            QT = [qtp.tile([128, T], F32R, tag=f"QT{h}", name=f"QT{h}") for h in range(LQH)]

            def rope_from_psum(psum, dst_slice, tqs, work):
                """dst = psum*cos + rot_half(psum)*sin_signed on one [128,TQW] tile."""
                raw = work.tile([128, TQW], F32, tag="rraw")
                nc.vector.tensor_copy(raw, psum)
                swp = work.tile([128, TQW], F32, tag="rswp")
                nc.gpsimd.dma_start(out=swp[0:64, :], in_=raw[64:128, :])
                nc.gpsimd.dma_start(out=swp[64:128, :], in_=raw[0:64, :])
                t1 = work.tile([128, TQW], F32, tag="rt1")
                nc.vector.tensor_mul(t1, raw, cos_t[:, tqs])
                t2 = work.tile([128, TQW], F32, tag="rt2")
                nc.vector.tensor_mul(t2, swp, sin_t[:, tqs])
                nc.vector.tensor_add(dst_slice, t1, t2)

            # ---------- Phase A1: K projection + rope + kv(k) ----------
            with tc.tile_pool(name="a1w", bufs=1) as a1w, \
                 tc.tile_pool(name="a1work", bufs=3) as a1work:
                wk_b = []
                for g8 in range(DC // 8):
                    t = a1w.tile([128, 8, EKV], F32R, tag=f"wkb{g8}",
                                 name=f"wkb{g8}")
                    nc.gpsimd.dma_start(
                        out=t, in_=wkg[g8 * 1024:(g8 + 1) * 1024, :]
                        .rearrange("(c p) e -> p c e", p=128))
                    wk_b.append(t)
                wk_t = [wk_b[c // 8][:, c % 8, :] for c in range(DC)]
                with tc.tile_pool(name="a1ps", bufs=1, space="PSUM") as a1ps:
                    pk = [[a1ps.tile([128, TQW], F32, tag=f"pk{tq}{h}",
                                     name=f"pk{tq}{h}") for h in range(LKVH)]
                          for tq in range(NTQ)]
                    for c in range(DC):
                        ktile = a1work.tile([128, T], F32R, tag="ktile")
                        eng = nc.sync if c % 2 == 0 else nc.scalar
                        eng.dma_start(out=ktile,
                                      in_=kT[c * 128:(c + 1) * 128, :])
                        for tq in range(NTQ):
                            for h in range(LKVH):
                                nc.tensor.matmul(
                                    pk[tq][h],
                                    wk_t[c][:, h * HD:(h + 1) * HD],
                                    ktile[:, tq * TQW:(tq + 1) * TQW],
                                    start=(c == 0), stop=(c == DC - 1))
                    for tq in range(NTQ):
                        tqs = slice(tq * TQW, (tq + 1) * TQW)
                        for h in range(LKVH):
                            rope_from_psum(pk[tq][h], KT[h][:, tqs], tqs,
                                           a1work)
                with tc.tile_pool(name="a1tp", bufs=2, space="PSUM") as a1tp:
                    for tq in range(NTQ):
                        tqs = slice(tq * TQW, (tq + 1) * TQW)
                        for h in range(LKVH):
                            ksb = a1work.tile([128, TQW // 128, 128], F32,
                                              tag="ksb")
                            for j in range(TQW // 128):
                                tt = tq * TQW + j * 128
                                tp = a1tp.tile([128, 128], F32, tag="ktp")
                                nc.tensor.transpose(
                                    tp, KT[h][:, tt:tt + 128].bitcast(F32),
                                    ident)
                                nc.scalar.copy(ksb[:, j, :], tp)
                            nc.scalar.dma_start(
                                out=kvout[tqs, h, 0:HD].rearrange(
                                    "(j p) d -> p j d", p=128),
                                in_=ksb)

            # ---------- Phase A2: V projection + kv(v) ----------
            with tc.tile_pool(name="a2w", bufs=1) as a2w, \
                 tc.tile_pool(name="a2ps", bufs=1, space="PSUM") as a2ps, \
                 tc.tile_pool(name="a2work", bufs=3) as a2work:
                wv_b = []
                for g8 in range(DC // 8):
                    t = a2w.tile([128, 8, EKV], F32R, tag=f"wvb{g8}",
                                 name=f"wvb{g8}")
                    nc.gpsimd.dma_start(
                        out=t, in_=wvg[g8 * 1024:(g8 + 1) * 1024, :]
                        .rearrange("(c p) e -> p c e", p=128))
                    wv_b.append(t)
                wv_t = [wv_b[c // 8][:, c % 8, :] for c in range(DC)]
                for tcg in range(T // 1024):
                    gs = slice(tcg * 1024, (tcg + 1) * 1024)
                    pv = [a2ps.tile([128, EKV], F32, tag=f"pv{u}",
                                    name=f"pv{u}") for u in range(8)]
                    for c in range(DC):
                        vtile = a2work.tile([128, 1024], F32R, tag="vtile")
                        eng = nc.sync if c % 2 == 0 else nc.scalar
                        eng.dma_start(out=vtile,
                                      in_=vT[c * 128:(c + 1) * 128, gs])
                        for u in range(8):
                            nc.tensor.matmul(pv[u],
                                             vtile[:, u * 128:(u + 1) * 128],
                                             wv_t[c], start=(c == 0),
                                             stop=(c == DC - 1))
                    for u in range(8):
                        tci = tcg * 8 + u
                        for h in range(LKVH):
                            nc.vector.tensor_copy(V[h][:, tci, :],
                                                  pv[u][:, h * HD:(h + 1) * HD])
                    for h in range(LKVH):
                        nc.scalar.dma_start(
                            out=kvout[gs, h, HD:2 * HD].rearrange(
                                "(j p) d -> p j d", p=128),
                            in_=V[h][:, tcg * 8:(tcg + 1) * 8, :].bitcast(F32))

            # ---------- Phase A3: Q projection + rope ----------
            CG = 4
            with tc.tile_pool(name="a3ps", bufs=1, space="PSUM") as a3ps, \
                 tc.tile_pool(name="a3work", bufs=3) as a3work, \
                 tc.tile_pool(name="a3w", bufs=2) as a3w:
                for tq in range(NTQ):
                    tqs = slice(tq * TQW, (tq + 1) * TQW)
                    pq = [a3ps.tile([128, TQW], F32, tag=f"pq{h}",
                                    name=f"pq{h}") for h in range(LQH)]
                    for cg in range(DC // CG):
                        qbig = a3work.tile([128, CG, TQW], F32R, tag="qbig")
                        enga = nc.sync if cg % 2 == 0 else nc.scalar
                        enga.dma_start(
                            out=qbig,
                            in_=qT[cg * CG * 128:(cg + 1) * CG * 128, tqs]
                            .rearrange("(c p) t -> p c t", p=128))
                        wqbig = a3w.tile([128, CG, EQ], F32R, tag="wqbig")
                        engb = nc.scalar if cg % 2 == 0 else nc.sync
                        engb.dma_start(
                            out=wqbig,
                            in_=wqg[cg * CG * 128:(cg + 1) * CG * 128, :]
                            .rearrange("(c p) e -> p c e", p=128))
                        for i in range(CG):
                            c = cg * CG + i
                            for h in range(LQH):
                                nc.tensor.matmul(
                                    pq[h], wqbig[:, i, h * HD:(h + 1) * HD],
                                    qbig[:, i, :], start=(c == 0),
                                    stop=(c == DC - 1))
                    for h in range(LQH):
                        rope_from_psum(pq[h], QT[h][:, tqs], tqs, a3work)

            tables_cm.__exit__(None, None, None)

            # ---------- Phase B: attention ----------
            otp = ctx.enter_context(tc.tile_pool(name="otp", bufs=1, side="right"))
            OT = [otp.tile([128, T], F32R, tag=f"OT{h}", name=f"OT{h}")
                  for h in range(LQH)]
            with tc.tile_pool(name="bsc", bufs=4, space="PSUM") as bsc, \
                 tc.tile_pool(name="bo", bufs=2, space="PSUM") as bo, \
                 tc.tile_pool(name="bden", bufs=2, space="PSUM") as bden, \
                 tc.tile_pool(name="bwork", bufs=5) as bwork, \
                 tc.tile_pool(name="bnorm", bufs=2) as bnorm:
                for tq in range(NTQ):
                    tqs = slice(tq * TQW, (tq + 1) * TQW)
                    nkc = (tq + 1) * (TQW // KCW)
                    for h2 in range(LKVH):
                        for gr in range(GRP):
                            qh = h2 * GRP + gr

                            def emit_sc(kc):
                                scp = bsc.tile([128, TQW], F32, tag="scp",
                                               name="scp")
                                nc.tensor.matmul(
                                    scp, KT[h2][:, kc * KCW:(kc + 1) * KCW],
                                    QT[qh][:, tqs], start=True, stop=True)
                                ext = bwork.tile([128, TQW], F32R, tag="ext",
                                                 name="ext")
                                nc.scalar.activation(
                                    ext, scp,
                                    mybir.ActivationFunctionType.Exp,
                                    bias=0.0, scale=SCALE)
                                j = kc - 4 * tq
                                if j >= 0:
                                    nc.vector.tensor_mul(ext, ext,
                                                         mask_t[:, j, :])
                                return ext

                            ops_ = bo.tile([128, TQW], F32, tag="ops",
                                           name="ops")
                            dps = bden.tile([1, TQW], F32, tag="dps",
                                            name="dps")
                            depth = 3
                            pend = [emit_sc(i) for i in range(min(depth, nkc))]
                            for kc in range(nkc):
                                ext = pend.pop(0)
                                if kc + depth < nkc:
                                    pend.append(emit_sc(kc + depth))
                                nc.tensor.matmul(ops_, V[h2][:, kc, :], ext,
                                                 start=(kc == 0),
                                                 stop=(kc == nkc - 1),
                                                 skip_group_check=True)
                                nc.tensor.matmul(dps, ones_c, ext,
                                                 start=(kc == 0),
                                                 stop=(kc == nkc - 1),
                                                 skip_group_check=True)
                            den_sb = bnorm.tile([1, TQW], F32, tag="den",
                                                name="den")
                            nc.vector.tensor_copy(den_sb, dps)
                            rec_sb = bnorm.tile([1, TQW], F32, tag="rec",
                                                name="rec")
                            nc.vector.reciprocal(rec_sb, den_sb)
                            bcs = bnorm.tile([128, TQW], F32, tag="bcs",
                                             name="bcs")
                            nc.gpsimd.partition_broadcast(bcs, rec_sb)
                            nc.vector.tensor_mul(OT[qh][:, tqs], ops_, bcs)

        # ---------- Phase C: wo ----------
        with tc.tile_pool(name="cw", bufs=2) as cw, \
             tc.tile_pool(name="cps", bufs=4, space="PSUM") as cps, \
             tc.tile_pool(name="cout", bufs=4) as cout:
            for nt in range(D // 512):
                nts = slice(nt * 512, (nt + 1) * 512)
                wo_t = []
                for e in range(LQH):
                    t = cw.tile([128, 512], F32R, tag=f"wo{e}", name=f"wo{e}")
                    nc.scalar.dma_start(out=t, in_=wog[e * 128:(e + 1) * 128, nts])
                    wo_t.append(t)
                for tci in range(T // 128):
                    ops = cps.tile([128, 512], F32, tag="cps")
                    for e in range(LQH):
                        nc.tensor.matmul(ops,
                                         OT[e][:, tci * 128:(tci + 1) * 128],
                                         wo_t[e], start=(e == 0),
                                         stop=(e == LQH - 1))
                    osb = cout.tile([128, 512], F32, tag="osb")
                    nc.scalar.copy(osb, ops)
                    nc.sync.dma_start(out=outp[tci * 128:(tci + 1) * 128, nts],
                                      in_=osb)

    nc.finalize()
    return nc


def _host_tables():
    exps = -np.arange(0, HD, 2, dtype=np.float64) / HD
    thetas = ROPE_BASE ** exps                       # [64]
    t = np.arange(T, dtype=np.float64)
    ticks = np.outer(thetas, t)                      # [64, T]
    cos_half = np.cos(ticks)
    sin_half = np.sin(ticks)
    cosdt = np.concatenate([cos_half, cos_half], 0).astype(np.float32)
    sindt = np.concatenate([-sin_half, sin_half], 0).astype(np.float32)

    p = np.arange(KCW)[:, None, None]
    j = np.arange(GRP)[None, :, None]
    c = np.arange(TQW)[None, None, :]
    mask4 = ((p + 128 * j) <= c).astype(np.float32)
    return cosdt, sindt, mask4


def kernel(q, k, v, wq, wk, wv, wo):
    global LAST_RESULTS
    q = np.asarray(q, np.float32)
    k = np.asarray(k, np.float32)
    v = np.asarray(v, np.float32)
    wq = np.asarray(wq, np.float32)
    wk = np.asarray(wk, np.float32)
    wv = np.asarray(wv, np.float32)
    wo = np.asarray(wo, np.float32)

    if "nc" not in _CACHE:
        _CACHE["nc"] = _build_nc()
    nc = _CACHE["nc"]

    cosdt, sindt, mask4 = _host_tables()
    ones_col = np.ones((128, 1), np.float32)
    ones_row = np.ones((1, 128), np.float32)
    identm = np.eye(128, dtype=np.float32)

    in_maps = []
    for core in range(8):
        b, g = core // 4, core % 4
        jlist = [8 * gr + 2 * g + h2 for h2 in range(LKVH) for gr in range(GRP)]
        cols = np.concatenate([np.arange(128 * j, 128 * (j + 1)) for j in jlist])
        in_maps.append({
            "qT": np.ascontiguousarray(q[b].T),
            "kT": np.ascontiguousarray(k[b].T),
            "vT": np.ascontiguousarray(v[b].T),
            "wqg": np.ascontiguousarray(wq[:, cols]),
            "wkg": np.ascontiguousarray(wk[:, 256 * g:256 * (g + 1)]),
            "wvg": np.ascontiguousarray(wv[:, 256 * g:256 * (g + 1)]),
            "wog": np.ascontiguousarray(wo[cols, :]),
            "cosdt": cosdt, "sindt": sindt, "mask4": mask4,
            "ones_col": ones_col, "ones_row": ones_row, "identm": identm,
        })

    trace = os.environ.get("TRN_KERNEL_TRACE") == "1"
    if trace:
        try:
            from trn_prof import install_ntff_hook
            install_ntff_hook()
        except Exception:
            trace = False
    res = run_bass_kernel_spmd(nc, in_maps, core_ids=list(range(8)), trace=trace)
    LAST_RESULTS = res

    out = np.zeros((B, T, D), np.float32)
    kv = np.zeros((B, T, NKVH, 2 * HD), np.float32)
    for core in range(8):
        b, g = core // 4, core % 4
        out[b] += res.results[core]["outp"]
        kvo = res.results[core]["kvout"]
        for h2 in range(LKVH):
            kv[b, :, 2 * g + h2, :] = kvo[:, h2, :]
    return out, kv


# revision 16
# speedup vs baseline: 1.0877x; 1.0877x over previous
"""TRN2 Bass kernel for nn_MultiHeadAttention_87119116632098 (GQA + RoPE + causal).

Sharding: batch (2) x kv-head-pair (4) = 8 cores, per the hint. Each core
computes Q/K/V projections for its 8 q-heads / 2 kv-heads, RoPE, causal
grouped attention, and a partial wo product (its 1024 wo input rows).
The host sums the 4 partial wo outputs per batch (the "all-reduce") and
reassembles the kv cache; both are exact fp32 adds / pure gathers.

All matmuls run as float32r (TRN2 reduced-precision fp32, ~13-bit
mantissa, 1 PE cycle/row at N>=256 vs 4 cycles/row for full fp32).
Measured end-to-end numerics vs the fp32 reference: ~1e-3 scale-relative.

Layouts (per core):
  qT/kT/vT   [D=4096, T=2048]  (host-transposed) -> moving/stationary chunks
  QT, KT     [head][128=headdim, T]   (projection output, transposed, roped)
  V          [head][128=t-chunk, 16, 128=headdim]  (natural)
  scores^T   [t_k=128, t_q=512] tiles; softmax without max-subtraction
             (scores are ~N(0,1.6), max ~10 -> exp is fp32-safe);
             denominators via ones-vector matmuls accumulated in PSUM,
             normalization via PE-broadcast of 1/denom.
  OT         [head][128=headdim, T]   (unnormalized attention out, f32r)
  out        [t=128, n=512] tiles = sum_e OT^T chunks @ wo chunks
"""
import os
import sys

sys.path.insert(0, "/opt/trn_rl_repo")

import numpy as np
from contextlib import ExitStack

import concourse.bass as bass
from concourse import bacc
import concourse.mybir as mybir
import concourse.tile as tile
from concourse.bass_utils import run_bass_kernel_spmd

B, T, D = 2, 2048, 4096
NQH, NKVH, HD = 32, 8, 128
GRP = NQH // NKVH              # 4
LQH, LKVH = 8, 2               # per-core q heads / kv heads
EQ = LQH * HD                  # 1024
EKV = LKVH * HD                # 256
DC = D // 128                  # 32 contraction chunks
TQW = 512                      # t_q tile width
NTQ = T // TQW                 # 4
KCW = 128                      # t_k chunk width
ROPE_BASE = 10000.0
SCALE = 1.0 / float(np.sqrt(HD))

F32 = mybir.dt.float32
F32R = mybir.dt.float32r

_CACHE = {}
LAST_RESULTS = None


def _build_nc():
    nc = bacc.Bacc("TRN2", target_bir_lowering=False, debug=True)

    qT = nc.dram_tensor("qT", [D, T], F32R, kind="ExternalInput")
    kT = nc.dram_tensor("kT", [D, T], F32R, kind="ExternalInput")
    vT = nc.dram_tensor("vT", [D, T], F32R, kind="ExternalInput")
    wqg = nc.dram_tensor("wqg", [D, EQ], F32R, kind="ExternalInput")
    wkg = nc.dram_tensor("wkg", [D, EKV], F32R, kind="ExternalInput")
    wvg = nc.dram_tensor("wvg", [D, EKV], F32R, kind="ExternalInput")
    wog = nc.dram_tensor("wog", [EQ, D], F32R, kind="ExternalInput")
    cosdt = nc.dram_tensor("cosdt", [HD, T], F32, kind="ExternalInput")
    sindt = nc.dram_tensor("sindt", [HD, T], F32, kind="ExternalInput")
    mask4 = nc.dram_tensor("mask4", [KCW, GRP, TQW], F32, kind="ExternalInput")
    ones_col = nc.dram_tensor("ones_col", [128, 1], F32R, kind="ExternalInput")
    ones_row = nc.dram_tensor("ones_row", [1, 128], F32R, kind="ExternalInput")
    identm = nc.dram_tensor("identm", [128, 128], F32, kind="ExternalInput")

    outp = nc.dram_tensor("outp", [T, D], F32, kind="ExternalOutput")
    kvout = nc.dram_tensor("kvout", [T, LKVH, 2 * HD], F32, kind="ExternalOutput")

    with tile.TileContext(nc) as tc, ExitStack() as ctx:
        consts = ctx.enter_context(tc.tile_pool(name="consts", bufs=1))
        ktv = ctx.enter_context(tc.tile_pool(name="ktv", bufs=1))

        ones_c = consts.tile([128, 1], F32R)
        nc.sync.dma_start(out=ones_c, in_=ones_col[:, :])
        ones_r = consts.tile([1, 128], F32R)
        nc.sync.dma_start(out=ones_r, in_=ones_row[:, :])
        ident = consts.tile([128, 128], F32)
        nc.sync.dma_start(out=ident, in_=identm[:, :])
        mask_t = consts.tile([KCW, GRP, TQW], F32)
        nc.sync.dma_start(out=mask_t, in_=mask4[:, :, :])

        KT = [ktv.tile([128, T], F32R, tag=f"KT{h}", name=f"KT{h}") for h in range(LKVH)]
        V = [ktv.tile([128, T // 128, HD], F32R, tag=f"V{h}", name=f"V{h}") for h in range(LKVH)]

        with tc.tile_pool(name="qtp", bufs=1) as qtp:
            tables_cm = tc.tile_pool(name="tables", bufs=1)
            tables = tables_cm.__enter__()
            cos_t = tables.tile([HD, T], F32)
            nc.sync.dma_start(out=cos_t, in_=cosdt[:, :])
            sin_t = tables.tile([HD, T], F32)
            nc.sync.dma_start(out=sin_t, in_=sindt[:, :])

            QT = [qtp.tile([128, T], F32R, tag=f"QT{h}", name=f"QT{h}") for h in range(LQH)]

            def rope_from_psum(psum, dst_slice, tqs, work):
                """dst = psum*cos + rot_half(psum)*sin_signed on one [128,TQW] tile."""
                raw = work.tile([128, TQW], F32, tag="rraw")
                nc.vector.tensor_copy(raw, psum)
                swp = work.tile([128, TQW], F32, tag="rswp")
                nc.gpsimd.dma_start(out=swp[0:64, :], in_=raw[64:128, :])
                nc.gpsimd.dma_start(out=swp[64:128, :], in_=raw[0:64, :])
                t1 = work.tile([128, TQW], F32, tag="rt1")
                nc.vector.tensor_mul(t1, raw, cos_t[:, tqs])
                t2 = work.tile([128, TQW], F32, tag="rt2")
                nc.vector.tensor_mul(t2, swp, sin_t[:, tqs])
                nc.vector.tensor_add(dst_slice, t1, t2)

            # ---------- Phase A1: K projection + rope + kv(k) ----------
            with tc.tile_pool(name="a1w", bufs=1) as a1w, \
                 tc.tile_pool(name="a1work", bufs=3) as a1work:
                wk_b = []
                for g8 in range(DC // 8):
                    t = a1w.tile([128, 8, EKV], F32R, tag=f"wkb{g8}",
                                 name=f"wkb{g8}")
                    nc.gpsimd.dma_start(
                        out=t, in_=wkg[g8 * 1024:(g8 + 1) * 1024, :]
                        .rearrange("(c p) e -> p c e", p=128))
                    wk_b.append(t)
                wk_t = [wk_b[c // 8][:, c % 8, :] for c in range(DC)]
                with tc.tile_pool(name="a1ps", bufs=1, space="PSUM") as a1ps:
                    pk = [[a1ps.tile([128, TQW], F32, tag=f"pk{tq}{h}",
                                     name=f"pk{tq}{h}") for h in range(LKVH)]
                          for tq in range(NTQ)]
                    for c in range(DC):
                        ktile = a1work.tile([128, T], F32R, tag="ktile")
                        eng = nc.sync if c % 2 == 0 else nc.scalar
                        eng.dma_start(out=ktile,
                                      in_=kT[c * 128:(c + 1) * 128, :])
                        for tq in range(NTQ):
                            for h in range(LKVH):
                                nc.tensor.matmul(
                                    pk[tq][h],
                                    wk_t[c][:, h * HD:(h + 1) * HD],
                                    ktile[:, tq * TQW:(tq + 1) * TQW],
                                    start=(c == 0), stop=(c == DC - 1))
                    for tq in range(NTQ):
                        tqs = slice(tq * TQW, (tq + 1) * TQW)
                        for h in range(LKVH):
                            rope_from_psum(pk[tq][h], KT[h][:, tqs], tqs,
                                           a1work)

            # ---------- Phase A2: V projection + kv(v) ----------
            with tc.tile_pool(name="a2w", bufs=1) as a2w, \
                 tc.tile_pool(name="a2ps", bufs=1, space="PSUM") as a2ps, \
                 tc.tile_pool(name="a2work", bufs=3) as a2work:
                wv_b = []
                for g8 in range(DC // 8):
                    t = a2w.tile([128, 8, EKV], F32R, tag=f"wvb{g8}",
                                 name=f"wvb{g8}")
                    nc.gpsimd.dma_start(
                        out=t, in_=wvg[g8 * 1024:(g8 + 1) * 1024, :]
                        .rearrange("(c p) e -> p c e", p=128))
                    wv_b.append(t)
                wv_t = [wv_b[c // 8][:, c % 8, :] for c in range(DC)]
                for tcg in range(T // 1024):
                    gs = slice(tcg * 1024, (tcg + 1) * 1024)
                    pv = [a2ps.tile([128, EKV], F32, tag=f"pv{u}",
                                    name=f"pv{u}") for u in range(8)]
                    for c in range(DC):
                        vtile = a2work.tile([128, 1024], F32R, tag="vtile")
                        eng = nc.sync if c % 2 == 0 else nc.scalar
                        eng.dma_start(out=vtile,
                                      in_=vT[c * 128:(c + 1) * 128, gs])
                        for u in range(8):
                            nc.tensor.matmul(pv[u],
                                             vtile[:, u * 128:(u + 1) * 128],
                                             wv_t[c], start=(c == 0),
                                             stop=(c == DC - 1))
                    for u in range(8):
                        tci = tcg * 8 + u
                        for h in range(LKVH):
                            nc.vector.tensor_copy(V[h][:, tci, :],
                                                  pv[u][:, h * HD:(h + 1) * HD])
                    for h in range(LKVH):
                        nc.scalar.dma_start(
                            out=kvout[gs, h, HD:2 * HD].rearrange(
                                "(j p) d -> p j d", p=128),
                            in_=V[h][:, tcg * 8:(tcg + 1) * 8, :].bitcast(F32))

            # ---------- A1 tail: kv(k) transposes ----------
            with tc.tile_pool(name="a1tp", bufs=2, space="PSUM") as a1tp, \
                 tc.tile_pool(name="ktpw", bufs=3) as ktpw:
                for tq in range(NTQ):
                    tqs = slice(tq * TQW, (tq + 1) * TQW)
                    for h in range(LKVH):
                        ksb = ktpw.tile([128, TQW // 128, 128], F32,
                                        tag="ksb")
                        for j in range(TQW // 128):
                            tt = tq * TQW + j * 128
                            tp = a1tp.tile([128, 128], F32, tag="ktp")
                            nc.tensor.transpose(
                                tp, KT[h][:, tt:tt + 128].bitcast(F32),
                                ident)
                            nc.scalar.copy(ksb[:, j, :], tp)
                        nc.scalar.dma_start(
                            out=kvout[tqs, h, 0:HD].rearrange(
                                "(j p) d -> p j d", p=128),
                            in_=ksb)

            # ---------- Phase A3: Q projection + rope ----------
            CG = 4
            with tc.tile_pool(name="a3ps", bufs=1, space="PSUM") as a3ps, \
                 tc.tile_pool(name="a3work", bufs=3) as a3work, \
                 tc.tile_pool(name="a3w", bufs=2) as a3w:
                for tq in range(NTQ):
                    tqs = slice(tq * TQW, (tq + 1) * TQW)
                    pq = [a3ps.tile([128, TQW], F32, tag=f"pq{h}",
                                    name=f"pq{h}") for h in range(LQH)]
                    for cg in range(DC // CG):
                        qbig = a3work.tile([128, CG, TQW], F32R, tag="qbig")
                        enga = nc.sync if cg % 2 == 0 else nc.scalar
                        enga.dma_start(
                            out=qbig,
                            in_=qT[cg * CG * 128:(cg + 1) * CG * 128, tqs]
                            .rearrange("(c p) t -> p c t", p=128))
                        wqbig = a3w.tile([128, CG, EQ], F32R, tag="wqbig")
                        engb = nc.scalar if cg % 2 == 0 else nc.sync
                        engb.dma_start(
                            out=wqbig,
                            in_=wqg[cg * CG * 128:(cg + 1) * CG * 128, :]
                            .rearrange("(c p) e -> p c e", p=128))
                        for i in range(CG):
                            c = cg * CG + i
                            for h in range(LQH):
                                nc.tensor.matmul(
                                    pq[h], wqbig[:, i, h * HD:(h + 1) * HD],
                                    qbig[:, i, :], start=(c == 0),
                                    stop=(c == DC - 1))
                    for h in range(LQH):
                        rope_from_psum(pq[h], QT[h][:, tqs], tqs, a3work)

            tables_cm.__exit__(None, None, None)

            # ---------- Phase B: attention ----------
            otp = ctx.enter_context(tc.tile_pool(name="otp", bufs=1, side="right"))
            OT = [otp.tile([128, T], F32R, tag=f"OT{h}", name=f"OT{h}")
                  for h in range(LQH)]
            with tc.tile_pool(name="bsc", bufs=4, space="PSUM") as bsc, \
                 tc.tile_pool(name="bo", bufs=2, space="PSUM") as bo, \
                 tc.tile_pool(name="bden", bufs=2, space="PSUM") as bden, \
                 tc.tile_pool(name="bwork", bufs=5) as bwork, \
                 tc.tile_pool(name="bnorm", bufs=2) as bnorm:
                for tq in range(NTQ):
                    tqs = slice(tq * TQW, (tq + 1) * TQW)
                    nkc = (tq + 1) * (TQW // KCW)
                    for h2 in range(LKVH):
                        for gr in range(GRP):
                            qh = h2 * GRP + gr

                            def emit_sc(kc):
                                scp = bsc.tile([128, TQW], F32, tag="scp",
                                               name="scp")
                                nc.tensor.matmul(
                                    scp, KT[h2][:, kc * KCW:(kc + 1) * KCW],
                                    QT[qh][:, tqs], start=True, stop=True)
                                ext = bwork.tile([128, TQW], F32R, tag="ext",
                                                 name="ext")
                                nc.scalar.activation(
                                    ext, scp,
                                    mybir.ActivationFunctionType.Exp,
                                    bias=0.0, scale=SCALE)
                                j = kc - 4 * tq
                                if j >= 0:
                                    nc.vector.tensor_mul(ext, ext,
                                                         mask_t[:, j, :])
                                return ext

                            ops_ = bo.tile([128, TQW], F32, tag="ops",
                                           name="ops")
                            dps = bden.tile([1, TQW], F32, tag="dps",
                                            name="dps")
                            depth = 3
                            pend = [emit_sc(i) for i in range(min(depth, nkc))]
                            for kc in range(nkc):
                                ext = pend.pop(0)
                                if kc + depth < nkc:
                                    pend.append(emit_sc(kc + depth))
                                nc.tensor.matmul(ops_, V[h2][:, kc, :], ext,
                                                 start=(kc == 0),
                                                 stop=(kc == nkc - 1),
                                                 skip_group_check=True)
                                nc.tensor.matmul(dps, ones_c, ext,
                                                 start=(kc == 0),
                                                 stop=(kc == nkc - 1),
                                                 skip_group_check=True)
                            den_sb = bnorm.tile([1, TQW], F32, tag="den",
                                                name="den")
                            nc.scalar.copy(den_sb, dps)
                            rec_sb = bnorm.tile([1, TQW], F32, tag="rec",
                                                name="rec")
                            nc.vector.reciprocal(rec_sb, den_sb)
                            bcs = bnorm.tile([128, TQW], F32, tag="bcs",
                                             name="bcs")
                            nc.gpsimd.partition_broadcast(bcs, rec_sb)
                            nc.vector.tensor_mul(OT[qh][:, tqs], ops_, bcs)

        # ---------- Phase C: wo ----------
        with tc.tile_pool(name="cw", bufs=2) as cw, \
             tc.tile_pool(name="cps", bufs=4, space="PSUM") as cps, \
             tc.tile_pool(name="cout", bufs=4) as cout:
            for nt in range(D // 512):
                nts = slice(nt * 512, (nt + 1) * 512)
                wo_t = []
                for e in range(LQH):
                    t = cw.tile([128, 512], F32R, tag=f"wo{e}", name=f"wo{e}")
                    nc.scalar.dma_start(out=t, in_=wog[e * 128:(e + 1) * 128, nts])
                    wo_t.append(t)
                for tci in range(T // 128):
                    ops = cps.tile([128, 512], F32, tag="cps")
                    for e in range(LQH):
                        nc.tensor.matmul(ops,
                                         OT[e][:, tci * 128:(tci + 1) * 128],
                                         wo_t[e], start=(e == 0),
                                         stop=(e == LQH - 1))
                    osb = cout.tile([128, 512], F32, tag="osb")
                    nc.scalar.copy(osb, ops)
                    nc.sync.dma_start(out=outp[tci * 128:(tci + 1) * 128, nts],
                                      in_=osb)

    nc.finalize()
    return nc


def _host_tables():
    exps = -np.arange(0, HD, 2, dtype=np.float64) / HD
    thetas = ROPE_BASE ** exps                       # [64]
    t = np.arange(T, dtype=np.float64)
    ticks = np.outer(thetas, t)                      # [64, T]
    cos_half = np.cos(ticks)
    sin_half = np.sin(ticks)
    cosdt = np.concatenate([cos_half, cos_half], 0).astype(np.float32)
    sindt = np.concatenate([-sin_half, sin_half], 0).astype(np.float32)

    p = np.arange(KCW)[:, None, None]
    j = np.arange(GRP)[None, :, None]
    c = np.arange(TQW)[None, None, :]
    mask4 = ((p + 128 * j) <= c).astype(np.float32)
    return cosdt, sindt, mask4


def kernel(q, k, v, wq, wk, wv, wo):
    global LAST_RESULTS
    q = np.asarray(q, np.float32)
    k = np.asarray(k, np.float32)
    v = np.asarray(v, np.float32)
    wq = np.asarray(wq, np.float32)
    wk = np.asarray(wk, np.float32)
    wv = np.asarray(wv, np.float32)
    wo = np.asarray(wo, np.float32)

    if "nc" not in _CACHE:
        _CACHE["nc"] = _build_nc()
    nc = _CACHE["nc"]

    cosdt, sindt, mask4 = _host_tables()
    ones_col = np.ones((128, 1), np.float32)
    ones_row = np.ones((1, 128), np.float32)
    identm = np.eye(128, dtype=np.float32)

    in_maps = []
    for core in range(8):
        b, g = core // 4, core % 4
        jlist = [8 * gr + 2 * g + h2 for h2 in range(LKVH) for gr in range(GRP)]
        cols = np.concatenate([np.arange(128 * j, 128 * (j + 1)) for j in jlist])
        in_maps.append({
            "qT": np.ascontiguousarray(q[b].T),
            "kT": np.ascontiguousarray(k[b].T),
            "vT": np.ascontiguousarray(v[b].T),
            "wqg": np.ascontiguousarray(wq[:, cols]),
            "wkg": np.ascontiguousarray(wk[:, 256 * g:256 * (g + 1)]),
            "wvg": np.ascontiguousarray(wv[:, 256 * g:256 * (g + 1)]),
            "wog": np.ascontiguousarray(wo[cols, :]),
            "cosdt": cosdt, "sindt": sindt, "mask4": mask4,
            "ones_col": ones_col, "ones_row": ones_row, "identm": identm,
        })

    trace = os.environ.get("TRN_KERNEL_TRACE") == "1"
    if trace:
        try:
            from trn_prof import install_ntff_hook
            install_ntff_hook()
        except Exception:
            trace = False
    res = run_bass_kernel_spmd(nc, in_maps, core_ids=list(range(8)), trace=trace)
    LAST_RESULTS = res

    out = np.zeros((B, T, D), np.float32)
    kv = np.zeros((B, T, NKVH, 2 * HD), np.float32)
    for core in range(8):
        b, g = core // 4, core % 4
        out[b] += res.results[core]["outp"]
        kvo = res.results[core]["kvout"]
        for h2 in range(LKVH):
            kv[b, :, 2 * g + h2, :] = kvo[:, h2, :]
    return out, kv
